# revision 4
# baseline (speedup 1.0000x reference)
"""CrossCosineEmbeddingLoss kernel for 8 trn2 NeuronCores.

loss = mean over all (i,j) of: 1 - cos(x_i, y_j) if i==j else relu(cos(x_i, y_j))

Identity:  total = sum_ij relu(sim_ij) + sum_i (1 - sim_ii - relu(sim_ii))
Sharding: rows of x across 8 cores (1024 rows each); y replicated.

Per-core pipeline (v4):
  - y: SWDGE cast-DMA HBM fp32 -> SBUF bf16 natural tiles; per-tile DMA
    xbar transposes build yT [d, j] bf16 (no PE transposes at all)
  - sumsq of y rows for 1/||y_j||, applied to per-block row sums at the
    end (relu commutes with positive scaling)
  - x: sumsq -> 1/||x|| -> scale+cast to bf16 xhat -> DMA transpose xhatT
  - main: 64 j-tiles: 2 bf16 matmuls (yT tile stationary, FWL weight
    loads hide under the stream) -> [128,1024] fp32 PSUM -> fused
    relu+accum split across ACT and DVE
  - final: R * rny, reduce; diagonal correction from natural bf16 tiles
Host combines [128,2] partials from each core.
"""

import numpy as np

import concourse.bacc as bacc
import concourse.bass as bass
import concourse.tile as tile
from concourse import mybir
from concourse.bass_utils import run_bass_kernel_spmd

N, D = 8192, 128
NCORES = 8
SH = N // NCORES          # 1024 rows of x per core
TX = SH // 128            # 8 x-tiles per core
TY = N // 128             # 64 y-tiles
YG = 8                    # y load groups (8 tiles each)

f32 = mybir.dt.float32
bf16 = mybir.dt.bfloat16
AF = mybir.ActivationFunctionType
ALU = mybir.AluOpType
AX = mybir.AxisListType

# ---- tuning knobs
Y_SUMSQ_ENGINE = "dve"      # "gpsimd" | "dve"
ACT_TILES = 34              # of 64 main tiles handled by ACT (rest DVE)


def _reduce_kind(t):
    # spread ACT_TILES evenly over the 64 iterations
    return "act" if (t * ACT_TILES) % TY < ACT_TILES else "dve"


_CACHE = {}


def _build():
    if "nc" in _CACHE:
        return _CACHE["nc"]
    nc = bacc.Bacc("TRN2", target_bir_lowering=False, debug=False,
                   num_devices=NCORES)
    xs_d = nc.dram_tensor("xs", [SH, D], f32, kind="ExternalInput")
    y_d = nc.dram_tensor("y", [N, D], f32, kind="ExternalInput")
    yd_d = nc.dram_tensor("yd", [SH, D], f32, kind="ExternalInput")
    out_d = nc.dram_tensor("out", [128, 2], f32, kind="ExternalOutput")

    with tile.TileContext(nc) as tc:
        with (
            tc.tile_pool(name="singles", bufs=1) as singles,
            tc.tile_pool(name="scrD", bufs=2) as scrD,
        ):
            ybf = singles.tile([128, TY, 128], bf16)    # [j%128, jt, d]
            yT = singles.tile([128, TY, 128], bf16)     # [d, jt, j]
            xnat = singles.tile([128, TX, 128], f32)    # [i%128, it, d]
            ydbf = singles.tile([128, TX, 128], bf16)   # local y rows
            xhat = singles.tile([128, TX, 128], bf16)
            xhatT = singles.tile([128, TX, 128], bf16)  # [d, it, i]
            nx2 = singles.tile([128, TX], f32)
            rnx = singles.tile([128, TX], f32)
            ny2 = singles.tile([128, TY], f32)
            rny = singles.tile([128, TY], f32)
            t2y = singles.tile([128, TY], f32)
            t1x = singles.tile([128, TX], f32)
            R = singles.tile([128, TY], f32)
            Ssc = singles.tile([128, TY], f32)
            d2 = singles.tile([128, TX], f32)
            sim_d = singles.tile([128, TX], f32)
            relu_d = singles.tile([128, TX], f32)
            outsb = singles.tile([128, 2], f32)
            warm = singles.tile([128, 8], f32)

            # preload the sqrt activation table set early (overlaps DMA)
            nc.vector.memset(warm[:], 1.0)
            nc.scalar.sqrt(warm[:], warm[:])

            # ---- load x shard + local y rows: rows r = 128*t + p -> (p,t,d)
            nc.sync.dma_start(
                out=xnat[:], in_=xs_d[:].rearrange("(t p) d -> p t d", p=128))
            nc.gpsimd.dma_start(
                out=ydbf[:], in_=yd_d[:].rearrange("(t p) d -> p t d", p=128))

            # ---- x norms + scale+cast (DVE) then DMA-transpose
            for t in range(TX):
                nc.vector.scalar_tensor_tensor(
                    out=scrD.tile([128, 128], f32, tag='sd', name='sd')[:],
                    in0=xnat[:, t, :], scalar=1.0, in1=xnat[:, t, :],
                    op0=ALU.mult, op1=ALU.mult, accum_out=nx2[:, t:t + 1])
            nc.vector.reciprocal(t1x[:], nx2[:])
            nc.scalar.sqrt(rnx[:], t1x[:])   # 1/||x_r||
            for t in range(TX):
                nc.vector.tensor_scalar(
                    out=xhat[:, t, :], in0=xnat[:, t, :],
                    scalar1=rnx[:, t:t + 1], scalar2=None,
                    op0=ALU.mult, op1=ALU.bypass)
                nc.sync.dma_start_transpose(
                    out=xhatT[:, t, :], in_=xhat[:, t, :])

            # ---- y: cast-DMA groups, sumsq, DMA transpose
            sq_eng = nc.gpsimd if Y_SUMSQ_ENGINE == "gpsimd" else nc.vector
            for g in range(YG):
                gs = slice(g * TX, (g + 1) * TX)
                nc.gpsimd.dma_start(
                    out=ybf[:, gs, :],
                    in_=y_d[1024 * g:1024 * (g + 1), :]
                    .rearrange("(t p) d -> p t d", p=128))
                for k in range(TX):
                    col = g * TX + k
                    sq_eng.scalar_tensor_tensor(
                        out=scrD.tile([128, 128], bf16, tag='sq', name='sq')[:],
                        in0=ybf[:, col, :], scalar=1.0, in1=ybf[:, col, :],
                        op0=ALU.mult, op1=ALU.mult,
                        accum_out=ny2[:, col:col + 1])
                    nc.sync.dma_start_transpose(
                        out=yT[:, col, :], in_=ybf[:, col, :])

            # ---- rny = 1/||y_j||
            nc.vector.reciprocal(t2y[:], ny2[:])
            nc.scalar.sqrt(rny[:], t2y[:])

            # ---- main: per j-block bf16 matmuls + fused relu-accumulate
            with tc.tile_pool(name="mpsum", bufs=4, space="PSUM") as mpsum:
                rhs = xhatT[:].rearrange("p a b -> p (a b)")
                for t in range(TY):
                    ps = mpsum.tile([128, 1024], f32, tag="mp")
                    lhsT = yT[:, t, :]
                    nc.tensor.matmul(ps[:, 0:512], lhsT, rhs[:, 0:512])
                    nc.tensor.matmul(ps[:, 512:1024], lhsT, rhs[:, 512:1024])
                    if _reduce_kind(t) == "act":
                        nc.scalar.activation(
                            ps[:], ps[:], AF.Relu, accum_out=R[:, t:t + 1])
                    else:
                        nc.vector.tensor_scalar(
                            out=ps[:], in0=ps[:], scalar1=0.0, scalar2=None,
                            op0=ALU.max, op1=ALU.add,
                            accum_out=R[:, t:t + 1])

            # ---- diagonal: sim_ii for local rows (x row i <-> y row i)
            for t in range(TX):
                nc.vector.scalar_tensor_tensor(
                    out=scrD.tile([128, 128], bf16, tag='dg', name='dg')[:],
                    in0=xhat[:, t, :], scalar=1.0, in1=ydbf[:, t, :],
                    op0=ALU.mult, op1=ALU.mult, accum_out=d2[:, t:t + 1])
            for t in range(TX):
                nc.vector.scalar_tensor_tensor(
                    out=scrD.tile([128, 128], bf16, tag='dq', name='dq')[:],
                    in0=ydbf[:, t, :], scalar=1.0, in1=ydbf[:, t, :],
                    op0=ALU.mult, op1=ALU.mult, accum_out=t1x[:, t:t + 1])
            nc.vector.reciprocal(nx2[:], t1x[:])   # reuse nx2 as tmp
            nc.scalar.sqrt(rnx[:], nx2[:])         # rnx <- 1/||y_i|| local
            nc.vector.tensor_mul(sim_d[:], d2[:], rnx[:])
            nc.scalar.activation(relu_d[:], sim_d[:], AF.Relu)
            nc.vector.scalar_tensor_tensor(
                out=scrD.tile([128, TX], f32, tag='df', name='df')[:],
                in0=sim_d[:], scalar=1.0, in1=relu_d[:],
                op0=ALU.mult, op1=ALU.add, accum_out=outsb[:, 1:2])

            # ---- final: scale per-block sums by 1/||y_j|| and total
            nc.vector.tensor_mul(Ssc[:], R[:], rny[:])
            nc.vector.tensor_reduce(out=outsb[:, 0:1], in_=Ssc[:],
                                    axis=AX.X, op=ALU.add)
            nc.sync.dma_start(out=out_d[:], in_=outsb[:])

    nc.compile()
    _CACHE["nc"] = nc
    return nc


def _in_maps(x, y):
    maps = []
    for c in range(NCORES):
        sl = slice(SH * c, SH * (c + 1))
        maps.append({"xs": np.ascontiguousarray(x[sl]),
                     "y": y,
                     "yd": np.ascontiguousarray(y[sl])})
    return maps


def _combine(results):
    total = 0.0
    for c in range(NCORES):
        o = results[c]["out"].astype(np.float64)
        total += o[:, 0].sum() - o[:, 1].sum() + SH
    return np.float32(total / (float(N) * float(N)))


def _run(x, y, trace=False):
    nc = _build()
    res = run_bass_kernel_spmd(nc, _in_maps(x, y), list(range(NCORES)),
                               trace=trace)
    return _combine(res.results), res


def kernel(x, y):
    x = np.asarray(x, dtype=np.float32)
    y = np.asarray(y, dtype=np.float32)
    loss, _ = _run(x, y, trace=False)
    return loss


# revision 5
# speedup vs baseline: 1.5863x; 1.5863x over previous
"""CrossCosineEmbeddingLoss kernel for 8 trn2 NeuronCores.

loss = mean over all (i,j) of: 1 - cos(x_i, y_j) if i==j else relu(cos(x_i, y_j))

Identity:  total = sum_ij relu(sim_ij) + sum_i (1 - sim_ii - relu(sim_ii))
Sharding: rows of x across 8 cores (1024 rows each); y replicated.

Per-core pipeline (v5):
  - y: SWDGE cast-DMA HBM fp32 -> DRAM bf16 scratch; per-1024-row-group
    xbar transpose-DMA (sprays across all 16 SDMA engines) builds
    yT [d, j] bf16; natural bf16 tiles loaded from the scratch for sumsq
  - sumsq of y rows for 1/||y_j||, applied to per-block row sums at the
    end (relu commutes with positive scaling)
  - x: sumsq -> 1/||x|| -> scale+cast bf16 -> DRAM bounce -> transpose
  - main: 64 j-tiles: 2 bf16 matmuls (yT tile stationary, FWL weight
    loads hide under the stream) -> [128,1024] fp32 PSUM -> fused
    relu+accum split across ACT and DVE
  - final: R * rny, reduce; diagonal correction from natural bf16 tiles
Host combines [128,2] partials from each core.
"""

import numpy as np

import concourse.bacc as bacc
import concourse.bass as bass
import concourse.tile as tile
from concourse import mybir
from concourse.bass_utils import run_bass_kernel_spmd

N, D = 8192, 128
NCORES = 8
SH = N // NCORES          # 1024 rows of x per core
TX = SH // 128            # 8 x-tiles per core
TY = N // 128             # 64 y-tiles
YG = 8                    # y load groups (8 tiles each)

f32 = mybir.dt.float32
bf16 = mybir.dt.bfloat16
AF = mybir.ActivationFunctionType
ALU = mybir.AluOpType
AX = mybir.AxisListType

ACT_TILES = 34              # of 64 main tiles handled by ACT (rest DVE)


def _reduce_kind(t):
    # spread ACT_TILES evenly over the 64 iterations
    return "act" if (t * ACT_TILES) % TY < ACT_TILES else "dve"


_CACHE = {}


def _build():
    if "nc" in _CACHE:
        return _CACHE["nc"]
    nc = bacc.Bacc("TRN2", target_bir_lowering=False, debug=False,
                   num_devices=NCORES)
    xs_d = nc.dram_tensor("xs", [SH, D], f32, kind="ExternalInput")
    y_d = nc.dram_tensor("y", [N, D], f32, kind="ExternalInput")
    yd_d = nc.dram_tensor("yd", [SH, D], f32, kind="ExternalInput")
    out_d = nc.dram_tensor("out", [128, 2], f32, kind="ExternalOutput")
    ybf_d = nc.dram_tensor("ybf_scr", [N, D], bf16, kind="Internal")
    xh_d = nc.dram_tensor("xh_scr", [SH, D], bf16, kind="Internal")

    with tile.TileContext(nc) as tc:
        with (
            tc.tile_pool(name="singles", bufs=1) as singles,
            tc.tile_pool(name="scrD", bufs=2) as scrD,
        ):
            ybf = singles.tile([128, TY, 128], bf16)    # [j%128, jt, d]
            yT = singles.tile([128, TY, 128], bf16)     # [d, jt, j]
            xnat = singles.tile([128, TX, 128], f32)    # [i%128, it, d]
            ydbf = singles.tile([128, TX, 128], bf16)   # local y rows
            xhat = singles.tile([128, TX, 128], bf16)
            xhatT = singles.tile([128, TX, 128], bf16)  # [d, it, i]
            nx2 = singles.tile([128, TX], f32)
            rnx = singles.tile([128, TX], f32)
            ny2 = singles.tile([128, TY], f32)
            rny = singles.tile([128, TY], f32)
            t2y = singles.tile([128, TY], f32)
            t1x = singles.tile([128, TX], f32)
            R = singles.tile([128, TY], f32)
            Ssc = singles.tile([128, TY], f32)
            d2 = singles.tile([128, TX], f32)
            sim_d = singles.tile([128, TX], f32)
            relu_d = singles.tile([128, TX], f32)
            outsb = singles.tile([128, 2], f32)
            warm = singles.tile([128, 8], f32)

            # preload the sqrt activation table set early (overlaps DMA)
            nc.vector.memset(warm[:], 1.0)
            nc.scalar.sqrt(warm[:], warm[:])

            # ---- load x shard + local y rows: rows r = 128*t + p -> (p,t,d)
            nc.sync.dma_start(
                out=xnat[:], in_=xs_d[:].rearrange("(t p) d -> p t d", p=128))
            nc.gpsimd.dma_start(
                out=ydbf[:], in_=yd_d[:].rearrange("(t p) d -> p t d", p=128))

            # ---- y: cast to bf16 scratch, big transposes, natural + sumsq
            for g in range(YG):
                rows = slice(1024 * g, 1024 * (g + 1))
                gs = slice(g * TX, (g + 1) * TX)
                nc.gpsimd.dma_start(out=ybf_d[rows], in_=y_d[rows])
                nc.sync.dma_start_transpose(
                    out=yT[:, gs, :].rearrange("p t j -> p (t j)"),
                    in_=ybf_d[rows])
                nc.sync.dma_start(
                    out=ybf[:, gs, :],
                    in_=ybf_d[rows].rearrange("(t p) d -> p t d", p=128))
                for k in range(TX):
                    col = g * TX + k
                    nc.vector.scalar_tensor_tensor(
                        out=scrD.tile([128, 128], bf16, tag='sq', name='sq')[:],
                        in0=ybf[:, col, :], scalar=1.0, in1=ybf[:, col, :],
                        op0=ALU.mult, op1=ALU.mult,
                        accum_out=ny2[:, col:col + 1])

            # ---- x norms + scale+cast (DVE) then DRAM-bounce transpose
            for t in range(TX):
                nc.vector.scalar_tensor_tensor(
                    out=scrD.tile([128, 128], f32, tag='sd', name='sd')[:],
                    in0=xnat[:, t, :], scalar=1.0, in1=xnat[:, t, :],
                    op0=ALU.mult, op1=ALU.mult, accum_out=nx2[:, t:t + 1])
            nc.vector.reciprocal(t1x[:], nx2[:])
            nc.scalar.sqrt(rnx[:], t1x[:])   # 1/||x_r||
            for t in range(TX):
                nc.vector.tensor_scalar(
                    out=xhat[:, t, :], in0=xnat[:, t, :],
                    scalar1=rnx[:, t:t + 1], scalar2=None,
                    op0=ALU.mult, op1=ALU.bypass)
            nc.sync.dma_start(
                out=xh_d[:].rearrange("(t p) d -> p t d", p=128), in_=xhat[:])
            nc.sync.dma_start_transpose(
                out=xhatT[:].rearrange("p t i -> p (t i)"), in_=xh_d[:])

            # ---- rny = 1/||y_j||
            nc.vector.reciprocal(t2y[:], ny2[:])
            nc.scalar.sqrt(rny[:], t2y[:])

            # ---- main: per j-block bf16 matmuls + fused relu-accumulate
            with tc.tile_pool(name="mpsum", bufs=4, space="PSUM") as mpsum:
                rhs = xhatT[:].rearrange("p a b -> p (a b)")
                for t in range(TY):
                    ps = mpsum.tile([128, 1024], f32, tag="mp")
                    lhsT = yT[:, t, :]
                    nc.tensor.matmul(ps[:, 0:512], lhsT, rhs[:, 0:512])
                    nc.tensor.matmul(ps[:, 512:1024], lhsT, rhs[:, 512:1024])
                    if _reduce_kind(t) == "act":
                        nc.scalar.activation(
                            ps[:], ps[:], AF.Relu, accum_out=R[:, t:t + 1])
                    else:
                        nc.vector.tensor_scalar(
                            out=ps[:], in0=ps[:], scalar1=0.0, scalar2=None,
                            op0=ALU.max, op1=ALU.add,
                            accum_out=R[:, t:t + 1])

            # ---- diagonal: sim_ii for local rows (x row i <-> y row i)
            for t in range(TX):
                nc.vector.scalar_tensor_tensor(
                    out=scrD.tile([128, 128], bf16, tag='dg', name='dg')[:],
                    in0=xhat[:, t, :], scalar=1.0, in1=ydbf[:, t, :],
                    op0=ALU.mult, op1=ALU.mult, accum_out=d2[:, t:t + 1])
            for t in range(TX):
                nc.vector.scalar_tensor_tensor(
                    out=scrD.tile([128, 128], bf16, tag='dq', name='dq')[:],
                    in0=ydbf[:, t, :], scalar=1.0, in1=ydbf[:, t, :],
                    op0=ALU.mult, op1=ALU.mult, accum_out=t1x[:, t:t + 1])
            nc.vector.reciprocal(nx2[:], t1x[:])   # reuse nx2 as tmp
            nc.scalar.sqrt(rnx[:], nx2[:])         # rnx <- 1/||y_i|| local
            nc.vector.tensor_mul(sim_d[:], d2[:], rnx[:])
            nc.scalar.activation(relu_d[:], sim_d[:], AF.Relu)
            nc.vector.scalar_tensor_tensor(
                out=scrD.tile([128, TX], f32, tag='df', name='df')[:],
                in0=sim_d[:], scalar=1.0, in1=relu_d[:],
                op0=ALU.mult, op1=ALU.add, accum_out=outsb[:, 1:2])

            # ---- final: scale per-block sums by 1/||y_j|| and total
            nc.vector.tensor_mul(Ssc[:], R[:], rny[:])
            nc.vector.tensor_reduce(out=outsb[:, 0:1], in_=Ssc[:],
                                    axis=AX.X, op=ALU.add)
            nc.sync.dma_start(out=out_d[:], in_=outsb[:])

    nc.compile()
    _CACHE["nc"] = nc
    return nc


def _in_maps(x, y):
    maps = []
    for c in range(NCORES):
        sl = slice(SH * c, SH * (c + 1))
        maps.append({"xs": np.ascontiguousarray(x[sl]),
                     "y": y,
                     "yd": np.ascontiguousarray(y[sl])})
    return maps


def _combine(results):
    total = 0.0
    for c in range(NCORES):
        o = results[c]["out"].astype(np.float64)
        total += o[:, 0].sum() - o[:, 1].sum() + SH
    return np.float32(total / (float(N) * float(N)))


def _run(x, y, trace=False):
    nc = _build()
    res = run_bass_kernel_spmd(nc, _in_maps(x, y), list(range(NCORES)),
                               trace=trace)
    return _combine(res.results), res


def kernel(x, y):
    x = np.asarray(x, dtype=np.float32)
    y = np.asarray(y, dtype=np.float32)
    loss, _ = _run(x, y, trace=False)
    return loss
